# revision 1
# baseline (speedup 1.0000x reference)
"""MoE routing mixture kernel for Trainium2 (8 NeuronCores, SPMD data-parallel).

Math: out[b] = sum_k selection_score[b, idx[b,k]] * all_weight[idx[b,k]]
Rewritten as a dense matmul: out = C @ W_flat, where
  C[b,e]    = selection_score[b,e] * |{k : idx[b,k]==e}|      ([2048, 64])
  W_flat    = all_weight.reshape(64, 16384)
Sharding: batch rows split across 8 cores (256 rows each); W replicated.

Raw Bass (no Tile): this toolchain's descriptors carry at most one sync wait
and one sync update each, so all synchronization is standalone wait_ge
instructions plus .then_inc updates, one per instruction.

Pipeline per core:
  SP   : 6 small input DMAs -> 4 W-chunk DMAs -> 16 output DMAs (1 MiB each)
  DVE  : C = score * count(idx==e) per 128-row chunk; C^T copies from PSUM
  PE   : 2 transposes (C -> C^T), then 64 matmuls [64x128]@[64x512] -> PSUM
  ACT  : 64 PSUM->SBUF copies into 16 staging tiles (no slot reuse)
"""

import sys
from contextlib import ExitStack

import numpy as np

sys.path.insert(0, "/opt/trn_rl_repo")

BS, E, TOPK, PL, D = 2048, 64, 8, 32, 512
NF = PL * D  # 16384 flattened prompt*dim
N_CORES = 8
RPC = BS // N_CORES  # 256 rows per core
RCHUNKS = RPC // 128  # 2 row chunks of 128
HALF = NF // 2  # 8192: W stored on-chip as [128, 8192]
WCHUNKS = 8  # W loaded in 8 chunks of [128, 1024]
WCW = HALF // WCHUNKS  # 2048
SLICES = WCW // D  # 4 matmuls (512 cols) per (chunk, half)
NPSUM = 6  # matmul PSUM ring
NGRP = WCHUNKS * RCHUNKS * 2  # 16 staging groups of [128, 2048]

_cache: dict = {}


def _build_program():
    import concourse.bass as bass
    import concourse.mybir as mybir

    f32 = mybir.dt.float32
    nc = bass.Bass()

    scores_d = nc.declare_dram_parameter("scores", [RPC, E], f32, isOutput=False)
    idx_d = nc.declare_dram_parameter("idxf", [RPC, TOPK], f32, isOutput=False)
    # W_flat [64, 16384] host-rearranged to [128, 8192]:
    # partition h*64+e holds cols [h*8192, (h+1)*8192) of expert e.
    wk_d = nc.declare_dram_parameter("wk", [128, HALF], f32, isOutput=False)
    iota_d = nc.declare_dram_parameter("iota", [128, E], f32, isOutput=False)
    ident_d = nc.declare_dram_parameter("ident", [128, 128], f32, isOutput=False)
    out_d = nc.declare_dram_parameter("out", [RPC, NF], f32, isOutput=True)

    ctx = ExitStack()
    with ctx:
        f32r = mybir.dt.float32r
        sb = lambda shape, tag, dt=f32: ctx.enter_context(  # noqa: E731
            nc.sbuf_tensor(tag, shape, dt)
        )
        w_t = sb([128, HALF], "w_t")
        iota_t = sb([128, E], "iota_t")
        ident_t = sb([128, 128], "ident_t")
        sc_t = [sb([128, E], f"sc{r}") for r in range(RCHUNKS)]
        idx_t = [sb([128, TOPK], f"idx{r}") for r in range(RCHUNKS)]
        eqs = [sb([128, E], f"eq{i}") for i in range(TOPK)]
        prs = [sb([128, E], f"pr{i}") for i in range(TOPK // 2)]
        qds = [sb([128, E], f"qd{i}") for i in range(TOPK // 4)]
        cnt = [sb([128, E], f"cnt{r}") for r in range(RCHUNKS)]
        ct = [sb([128, 128], f"ct{r}") for r in range(RCHUNKS)]
        stg = [sb([128, WCW], f"stg{g}") for g in range(NGRP)]

        ctp = [
            ctx.enter_context(nc.psum_tensor(f"ctp{r}", [E, 128], f32))
            for r in range(RCHUNKS)
        ]
        pmm = [
            ctx.enter_context(nc.psum_tensor(f"pmm{i}", [128, D], f32))
            for i in range(NPSUM)
        ]

        s_in = ctx.enter_context(nc.semaphore("s_in"))
        s_w = ctx.enter_context(nc.semaphore("s_w"))
        s_dve = ctx.enter_context(nc.semaphore("s_dve"))
        s_pe = ctx.enter_context(nc.semaphore("s_pe"))
        s_act = ctx.enter_context(nc.semaphore("s_act"))
        s_cpv = ctx.enter_context(nc.semaphore("s_cpv"))
        s_out = ctx.enter_context(nc.semaphore("s_out"))

        # matmul m (PE order) -> (wchunk c, rowchunk rc, half h, slice s)
        def mm_seq():
            m = 0
            for c in range(WCHUNKS):
                for rc in range(RCHUNKS):
                    for h in range(2):
                        for s in range(SLICES):
                            yield m, c, rc, h, s
                            m += 1

        N_MM = WCHUNKS * RCHUNKS * 2 * SLICES  # 64

        block = ctx.enter_context(nc.Block())

        @block.sync
        def _(sp):
            sp.dma_start(out=iota_t[:], in_=iota_d[:]).then_inc(s_in, 16)
            sp.dma_start(out=ident_t[:], in_=ident_d[:]).then_inc(s_in, 16)
            for r in range(RCHUNKS):
                rows = slice(r * 128, (r + 1) * 128)
                sp.dma_start(out=sc_t[r][:], in_=scores_d[rows, :]).then_inc(s_in, 16)
                sp.dma_start(out=idx_t[r][:], in_=idx_d[rows, :]).then_inc(s_in, 16)
            for c in range(WCHUNKS):
                cols = slice(c * WCW, (c + 1) * WCW)
                sp.dma_start(out=w_t[:, cols], in_=wk_d[:, cols]).then_inc(s_w, 16)

        @block.vector
        def _(v):
            v.wait_ge(s_in, 96)
            for r in range(RCHUNKS):
                for k in range(TOPK):
                    v.tensor_scalar(
                        eqs[k][:],
                        iota_t[:],
                        idx_t[r][:, k : k + 1],
                        None,
                        mybir.AluOpType.is_equal,
                    )
                v.drain()
                for i in range(TOPK // 2):
                    v.tensor_add(prs[i][:], eqs[2 * i][:], eqs[2 * i + 1][:])
                v.drain()
                for i in range(TOPK // 4):
                    v.tensor_add(qds[i][:], prs[2 * i][:], prs[2 * i + 1][:])
                v.drain()
                v.tensor_add(cnt[r][:], qds[0][:], qds[1][:])
                v.drain()
                v.tensor_mul(cnt[r][:], cnt[r][:], sc_t[r][:]).then_inc(s_dve, 1)
            for r in range(RCHUNKS):
                v.wait_ge(s_pe, r + 1)
                v.tensor_copy(ct[r][:E, :], ctp[r][:]).then_inc(s_dve, 1)
                v.tensor_copy(ct[r][E:, :], ctp[r][:]).then_inc(s_dve, 1)
            # odd-m PSUM->SBUF copies (evens go to ACT)
            for m, c, rc, h, s in mm_seq():
                if m % 2 == 0:
                    continue
                v.wait_ge(s_pe, RCHUNKS + m + 1)
                gi = c * (RCHUNKS * 2) + rc * 2 + h
                v.tensor_copy(
                    stg[gi][:, s * D : (s + 1) * D], pmm[m % NPSUM][:]
                ).then_inc(s_cpv, 1)

        @block.tensor
        def _(t):
            t.wait_ge(s_in, 96)  # ident
            for r in range(RCHUNKS):
                t.wait_ge(s_dve, r + 1)
                t.transpose(ctp[r][:], cnt[r][:], ident_t[:]).then_inc(s_pe, 1)
            t.wait_ge(s_dve, RCHUNKS + 2 * RCHUNKS)  # all ct copies done
            cur_c = -1
            for m, c, rc, h, s in mm_seq():
                if c != cur_c:
                    t.wait_ge(s_w, 16 * (c + 1))
                    cur_c = c
                if m >= NPSUM:
                    mm = m - NPSUM
                    if mm % 2 == 0:
                        t.wait_ge(s_act, mm // 2 + 1)
                    else:
                        t.wait_ge(s_cpv, mm // 2 + 1)
                pslice = slice(h * E, (h + 1) * E)
                wc = c * WCW + s * D
                t.matmul(
                    pmm[m % NPSUM][:],
                    ct[rc][pslice, :],
                    w_t[pslice, wc : wc + D],
                    start=True,
                    stop=True,
                ).then_inc(s_pe, 1)

        @block.scalar
        def _(a):
            for m, c, rc, h, s in mm_seq():
                if m % 2 == 1:
                    continue
                a.wait_ge(s_pe, RCHUNKS + m + 1)
                gi = c * (RCHUNKS * 2) + rc * 2 + h
                a.copy(
                    stg[gi][:, s * D : (s + 1) * D], pmm[m % NPSUM][:]
                ).then_inc(s_act, 1)

        @block.gpsimd
        def _(gp):
            # Output stores on SWDGE: group gi ready when its 2 ACT + 2 DVE
            # copies are done.
            gi = 0
            for c in range(WCHUNKS):
                for rc in range(RCHUNKS):
                    for h in range(2):
                        rows = slice(rc * 128, (rc + 1) * 128)
                        colbase = h * HALF + c * WCW
                        gp.wait_ge(s_act, (SLICES // 2) * (gi + 1))
                        gp.wait_ge(s_cpv, (SLICES // 2) * (gi + 1))
                        gp.dma_start(
                            out=out_d[rows, colbase : colbase + WCW],
                            in_=stg[gi][:],
                        ).then_inc(s_out, 16)
                        gi += 1
            gp.wait_ge(s_out, 16 * NGRP)

    return nc


def _run(selection_score, expert_indices, all_weight, trace=False):
    from concourse.bass_utils import run_bass_kernel_spmd

    scores = np.ascontiguousarray(np.asarray(selection_score, dtype=np.float32))
    idxf = np.ascontiguousarray(np.asarray(expert_indices).astype(np.float32))
    w = np.asarray(all_weight, dtype=np.float32).reshape(E, NF)
    wk = np.ascontiguousarray(
        w.reshape(E, 2, HALF).transpose(1, 0, 2).reshape(128, HALF)
    )
    iota = np.ascontiguousarray(np.tile(np.arange(E, dtype=np.float32), (128, 1)))
    ident = np.eye(128, dtype=np.float32)

    if "nc" not in _cache:
        _cache["nc"] = _build_program()
    nc = _cache["nc"]

    in_maps = [
        {
            "scores": np.ascontiguousarray(scores[c * RPC : (c + 1) * RPC]),
            "idxf": np.ascontiguousarray(idxf[c * RPC : (c + 1) * RPC]),
            "wk": wk,
            "iota": iota,
            "ident": ident,
        }
        for c in range(N_CORES)
    ]
    r = run_bass_kernel_spmd(nc, in_maps, list(range(N_CORES)), trace=trace)
    full = np.concatenate([r.results[c]["out"] for c in range(N_CORES)], axis=0)
    return full.reshape(BS, PL, D).astype(np.float32, copy=False), r


def kernel(selection_score, expert_indices, all_weight) -> np.ndarray:
    full, _ = _run(selection_score, expert_indices, all_weight, trace=False)
    return full



# revision 2
# speedup vs baseline: 1.9611x; 1.9611x over previous
"""MoE routing mixture kernel for Trainium2 (8 NeuronCores, SPMD data-parallel).

Math: out[b] = sum_k selection_score[b, idx[b,k]] * all_weight[idx[b,k]]
Rewritten as a dense matmul: out = C @ W_flat, where
  C[b,e]    = selection_score[b,e] * |{k : idx[b,k]==e}|      ([2048, 64])
  W_flat    = all_weight.reshape(64, 16384)
Sharding: batch rows split across 8 cores (256 rows each); W replicated.

The timeline cost model serializes all DMA transfers on one DMA_ENGINES
resource at ~360 B/ns, so makespan ~= bytes moved / 360 + issue/sem
overheads.  W is therefore loaded and the output stored in bf16, halving
the dominant traffic (20.5 MiB -> ~10 MiB per core); matmuls run in bf16
(1 PE cycle/row vs fp32's 4).  End-to-end rounding error ~4e-3 rel.

Raw Bass (no Tile): descriptors carry at most one sync wait and one sync
update each, so synchronization is standalone wait_ge instructions plus
.then_inc updates, one per instruction.

Pipeline per core (256 rows = 2 row chunks of 128):
  SP   : 1 fused aux DMA (iota|ident|scores|idx) -> 4 W-chunk DMAs (bf16,
         [128, 2048] cols each) -> 16 output DMAs ([128, 2048] bf16 each)
  DVE  : C = score * count(idx==e) per row chunk (bf16 eq/add tree);
         C^T copies from PSUM -> bf16; odd PSUM->SBUF copies
  PE   : 2 transposes (C -> C^T), then 64 bf16 matmuls [64x128]@[64x512]
  ACT  : even PSUM->SBUF copies (fp32 PSUM -> bf16 staging)
"""

import sys
from contextlib import ExitStack

import numpy as np

sys.path.insert(0, "/opt/trn_rl_repo")

BS, E, TOPK, PL, D = 2048, 64, 8, 32, 512
NF = PL * D  # 16384 flattened prompt*dim
N_CORES = 8
RPC = BS // N_CORES  # 256 rows per core
RCHUNKS = RPC // 128  # 2 row chunks of 128
HALF = NF // 2  # 8192: W stored on-chip as [128, 8192] bf16
WCHUNKS = 4  # W loaded in 4 chunks of [128, 2048]
WCW = HALF // WCHUNKS  # 2048
SLICES = WCW // D  # 4 matmuls (512 cols) per (chunk, rowchunk, half)
NPSUM = 6  # matmul PSUM ring
NGRP = WCHUNKS * RCHUNKS * 2  # 16 staging groups of [128, 2048] bf16

# aux tensor column layout (fp32): iota | ident | sc0 | sc1 | idx0 | idx1
A_IOTA = 0
A_IDENT = A_IOTA + E  # 64
A_SC = A_IDENT + 128  # 192
A_IDX = A_SC + RCHUNKS * E  # 320
A_COLS = A_IDX + RCHUNKS * TOPK  # 336

_cache: dict = {}


def _build_program():
    import concourse.bass as bass
    import concourse.mybir as mybir

    f32 = mybir.dt.float32
    bf16 = mybir.dt.bfloat16
    nc = bass.Bass()

    aux_d = nc.declare_dram_parameter("aux", [128, A_COLS], f32, isOutput=False)
    # W_flat [64, 16384] host-rearranged to [128, 8192] bf16:
    # partition h*64+e holds cols [h*8192, (h+1)*8192) of expert e.
    wk_d = nc.declare_dram_parameter("wk", [128, HALF], bf16, isOutput=False)
    out_d = nc.declare_dram_parameter("out", [RPC, NF], bf16, isOutput=True)

    ctx = ExitStack()
    with ctx:
        sb = lambda shape, tag, dt=f32: ctx.enter_context(  # noqa: E731
            nc.sbuf_tensor(tag, shape, dt)
        )
        aux_t = sb([128, A_COLS], "aux_t")
        iota_b = sb([128, E], "iota_b", bf16)
        w_t = sb([128, HALF], "w_t", bf16)
        eqs = [sb([128, E], f"eq{i}", bf16) for i in range(TOPK)]
        prs = [sb([128, E], f"pr{i}", bf16) for i in range(TOPK // 2)]
        qds = [sb([128, E], f"qd{i}", bf16) for i in range(TOPK // 4)]
        cnt = [sb([128, E], f"cnt{r}") for r in range(RCHUNKS)]
        ct = [sb([128, 128], f"ct{r}", bf16) for r in range(RCHUNKS)]
        stg = [sb([128, WCW], f"stg{g}", bf16) for g in range(NGRP)]

        ctp = [
            ctx.enter_context(nc.psum_tensor(f"ctp{r}", [E, 128], f32))
            for r in range(RCHUNKS)
        ]
        pmm = [
            ctx.enter_context(nc.psum_tensor(f"pmm{i}", [128, D], f32))
            for i in range(NPSUM)
        ]

        s_in = ctx.enter_context(nc.semaphore("s_in"))
        s_w = ctx.enter_context(nc.semaphore("s_w"))
        s_dve = ctx.enter_context(nc.semaphore("s_dve"))
        s_pe = ctx.enter_context(nc.semaphore("s_pe"))
        s_act = ctx.enter_context(nc.semaphore("s_act"))
        s_cpv = ctx.enter_context(nc.semaphore("s_cpv"))
        s_out = ctx.enter_context(nc.semaphore("s_out"))

        # matmul m (PE order) -> (wchunk c, rowchunk rc, half h, slice s)
        def mm_seq():
            m = 0
            for c in range(WCHUNKS):
                for rc in range(RCHUNKS):
                    for h in range(2):
                        for s in range(SLICES):
                            yield m, c, rc, h, s
                            m += 1

        ident = lambda: aux_t[:, A_IDENT : A_IDENT + 128]  # noqa: E731
        sc = lambda r: aux_t[:, A_SC + r * E : A_SC + (r + 1) * E]  # noqa: E731
        idxcol = lambda r, k: aux_t[  # noqa: E731
            :, A_IDX + r * TOPK + k : A_IDX + r * TOPK + k + 1
        ]

        block = ctx.enter_context(nc.Block())

        @block.sync
        def _(sp):
            sp.dma_start(out=aux_t[:], in_=aux_d[:]).then_inc(s_in, 16)
            for c in range(WCHUNKS):
                cols = slice(c * WCW, (c + 1) * WCW)
                sp.dma_start(out=w_t[:, cols], in_=wk_d[:, cols]).then_inc(s_w, 16)
            # Output stores: group gi=(c,rc,h) ready when its 2 ACT + 2 DVE
            # copies are done (copies alternate engines by matmul parity).
            for gi in range(NGRP):
                c, rc, h = gi // 4, (gi // 2) % 2, gi % 2
                rows = slice(rc * 128, (rc + 1) * 128)
                colbase = h * HALF + c * WCW
                sp.wait_ge(s_act, 2 * gi + 2)
                sp.wait_ge(s_cpv, 2 * gi + 2)
                sp.dma_start(
                    out=out_d[rows, colbase : colbase + WCW], in_=stg[gi][:]
                ).then_inc(s_out, 16)
            sp.wait_ge(s_out, 16 * NGRP)

        @block.vector
        def _(v):
            v.wait_ge(s_in, 16)
            v.tensor_copy(iota_b[:], aux_t[:, A_IOTA : A_IOTA + E])
            v.drain()
            for r in range(RCHUNKS):
                for k in range(TOPK):
                    v.tensor_scalar(
                        eqs[k][:],
                        iota_b[:],
                        idxcol(r, k),
                        None,
                        mybir.AluOpType.is_equal,
                    )
                v.drain()
                for i in range(TOPK // 2):
                    v.tensor_add(prs[i][:], eqs[2 * i][:], eqs[2 * i + 1][:])
                v.drain()
                for i in range(TOPK // 4):
                    v.tensor_add(qds[i][:], prs[2 * i][:], prs[2 * i + 1][:])
                v.drain()
                v.tensor_add(cnt[r][:], qds[0][:], qds[1][:])
                v.drain()
                v.tensor_mul(cnt[r][:], cnt[r][:], sc(r)).then_inc(s_dve, 1)
            for r in range(RCHUNKS):
                v.wait_ge(s_pe, r + 1)
                v.tensor_copy(ct[r][:E, :], ctp[r][:]).then_inc(s_dve, 1)
                v.tensor_copy(ct[r][E:, :], ctp[r][:]).then_inc(s_dve, 1)
            # odd-m PSUM->SBUF copies (evens go to ACT), fp32 -> bf16
            for m, c, rc, h, s in mm_seq():
                if m % 2 == 0:
                    continue
                v.wait_ge(s_pe, RCHUNKS + m + 1)
                gi = c * (RCHUNKS * 2) + rc * 2 + h
                v.tensor_copy(
                    stg[gi][:, s * D : (s + 1) * D], pmm[m % NPSUM][:]
                ).then_inc(s_cpv, 1)

        @block.tensor
        def _(t):
            t.wait_ge(s_in, 16)  # ident (aux)
            for r in range(RCHUNKS):
                t.wait_ge(s_dve, r + 1)
                t.transpose(ctp[r][:], cnt[r][:], ident()).then_inc(s_pe, 1)
            cur_c = -1
            for m, c, rc, h, s in mm_seq():
                if m == 0:
                    t.wait_ge(s_dve, RCHUNKS + 2)  # ct0 copies done
                if c == 0 and rc == 1 and h == 0 and s == 0:
                    t.wait_ge(s_dve, RCHUNKS + 4)  # ct1 copies done
                if c != cur_c:
                    t.wait_ge(s_w, 16 * (c + 1))
                    cur_c = c
                if m >= NPSUM:
                    mm = m - NPSUM
                    if mm % 2 == 0:
                        t.wait_ge(s_act, mm // 2 + 1)
                    else:
                        t.wait_ge(s_cpv, mm // 2 + 1)
                pslice = slice(h * E, (h + 1) * E)
                wc = c * WCW + s * D
                t.matmul(
                    pmm[m % NPSUM][:],
                    ct[rc][pslice, :],
                    w_t[pslice, wc : wc + D],
                    start=True,
                    stop=True,
                ).then_inc(s_pe, 1)

        @block.scalar
        def _(a):
            for m, c, rc, h, s in mm_seq():
                if m % 2 == 1:
                    continue
                a.wait_ge(s_pe, RCHUNKS + m + 1)
                gi = c * (RCHUNKS * 2) + rc * 2 + h
                a.copy(
                    stg[gi][:, s * D : (s + 1) * D], pmm[m % NPSUM][:]
                ).then_inc(s_act, 1)

    return nc


def _run(selection_score, expert_indices, all_weight, trace=False):
    import ml_dtypes
    from concourse.bass_utils import run_bass_kernel_spmd

    bf16 = ml_dtypes.bfloat16
    scores = np.asarray(selection_score, dtype=np.float32)
    idxf = np.asarray(expert_indices).astype(np.float32)
    w = np.asarray(all_weight, dtype=np.float32).reshape(E, NF)
    wk = np.ascontiguousarray(
        w.reshape(E, 2, HALF).transpose(1, 0, 2).reshape(128, HALF).astype(bf16)
    )
    iota = np.tile(np.arange(E, dtype=np.float32), (128, 1))
    ident = np.eye(128, dtype=np.float32)

    if "nc" not in _cache:
        _cache["nc"] = _build_program()
    nc = _cache["nc"]

    in_maps = []
    for c in range(N_CORES):
        rows = slice(c * RPC, (c + 1) * RPC)
        sc = scores[rows].reshape(RCHUNKS, 128, E)
        ix = idxf[rows].reshape(RCHUNKS, 128, TOPK)
        aux = np.concatenate(
            [iota, ident, sc[0], sc[1], ix[0], ix[1]], axis=1, dtype=np.float32
        )
        in_maps.append({"aux": np.ascontiguousarray(aux), "wk": wk})
    r = run_bass_kernel_spmd(nc, in_maps, list(range(N_CORES)), trace=trace)
    full = np.concatenate(
        [np.asarray(r.results[c]["out"]).astype(np.float32) for c in range(N_CORES)],
        axis=0,
    )
    return full.reshape(BS, PL, D), r


def kernel(selection_score, expert_indices, all_weight) -> np.ndarray:
    full, _ = _run(selection_score, expert_indices, all_weight, trace=False)
    return full


# revision 3
# speedup vs baseline: 1.9643x; 1.0017x over previous
"""MoE routing mixture kernel for Trainium2 (8 NeuronCores, SPMD data-parallel).

Math: out[b] = sum_k selection_score[b, idx[b,k]] * all_weight[idx[b,k]]
Rewritten as a dense matmul: out = C @ W_flat, where
  C[b,e]    = selection_score[b,e] * |{k : idx[b,k]==e}|      ([2048, 64])
  W_flat    = all_weight.reshape(64, 16384)
Sharding: batch rows split across 8 cores (256 rows each); W replicated.

The timeline cost model serializes all DMA transfers on one DMA_ENGINES
resource at ~360 B/ns, so makespan ~= bytes moved / 360 + issue/sem
overheads.  W is therefore loaded and the output stored in bf16, halving
the dominant traffic (20.5 MiB -> ~10 MiB per core); matmuls run in bf16
(1 PE cycle/row vs fp32's 4).  End-to-end rounding error ~4e-3 rel.

Raw Bass (no Tile): descriptors carry at most one sync wait and one sync
update each, so synchronization is standalone wait_ge instructions plus
.then_inc updates, one per instruction.

Head-latency optimizations (the store phase is DMA-back-to-back; the
makespan is set by when the first store's data is ready):
  - GPSIMD computes row-chunk 1's C = score*count chain in parallel with
    DVE's row-chunk 0 (GPSIMD runs fp32 at the same modeled rate).
  - PE warms up on scratch matmuls so the p-state ramp completes during
    the W-load phase and real matmuls run at full clock.
  - PE order: T0, c0/rc0 matmuls, T1, then the rest -- transpose1 (which
    needs GPSIMD's C1) doesn't block the first store group.
  - The first (c0, rc0) group is stored as 4x 1024-col DMAs instead of
    2x 2048 so the first store can start earlier.

Pipeline per core (256 rows = 2 row chunks of 128):
  SP   : fused aux DMA -> 4 W-chunk DMAs (bf16) -> 18 output DMAs (bf16)
  DVE  : rc0 C chain (bf16 eq/add tree); C^T copies from PSUM; odd
         PSUM->SBUF copies
  Pool : rc1 C chain (fp32)
  PE   : warmup, 2 transposes, 64 bf16 matmuls [64x128]@[64x512]
  ACT  : even PSUM->SBUF copies (fp32 PSUM -> bf16 staging)
"""

import sys
from contextlib import ExitStack

import numpy as np

sys.path.insert(0, "/opt/trn_rl_repo")

BS, E, TOPK, PL, D = 2048, 64, 8, 32, 512
NF = PL * D  # 16384 flattened prompt*dim
N_CORES = 8
RPC = BS // N_CORES  # 256 rows per core
RCHUNKS = RPC // 128  # 2 row chunks of 128
HALF = NF // 2  # 8192: W stored on-chip as [128, 8192] bf16
WCHUNKS = 4  # W loaded in 4 chunks of [128, 2048]
WCW = HALF // WCHUNKS  # 2048
SLICES = WCW // D  # 4 matmuls (512 cols) per (chunk, rowchunk, half)
NPSUM = 6  # matmul PSUM ring
N_WARM = 13  # PE warmup matmuls

# aux tensor column layout (fp32): iota | ident | sc0 | sc1 | idx0 | idx1
A_IOTA = 0
A_IDENT = A_IOTA + E  # 64
A_SC = A_IDENT + 128  # 192
A_IDX = A_SC + RCHUNKS * E  # 320
A_COLS = A_IDX + RCHUNKS * TOPK  # 336

_cache: dict = {}


def _build_program():
    import concourse.bass as bass
    import concourse.mybir as mybir

    f32 = mybir.dt.float32
    bf16 = mybir.dt.bfloat16
    nc = bass.Bass()

    aux_d = nc.declare_dram_parameter("aux", [128, A_COLS], f32, isOutput=False)
    # W_flat [64, 16384] host-rearranged to [128, 8192] bf16:
    # partition h*64+e holds cols [h*8192, (h+1)*8192) of expert e.
    wk_d = nc.declare_dram_parameter("wk", [128, HALF], bf16, isOutput=False)
    out_d = nc.declare_dram_parameter("out", [RPC, NF], bf16, isOutput=True)

    # matmul m (PE order) -> (wchunk c, rowchunk rc, half h, slice s)
    def mm_seq():
        m = 0
        for c in range(WCHUNKS):
            for rc in range(RCHUNKS):
                for h in range(2):
                    for s in range(SLICES):
                        yield m, c, rc, h, s
                        m += 1

    N_MM = WCHUNKS * RCHUNKS * 2 * SLICES  # 64

    # s_pe increment index of each matmul / transpose (PE order with
    # warmup first -- warmups don't touch s_pe; T1 sits after m=7).
    pe_idx: dict = {}
    n = 0
    n += 1
    pe_idx["T0"] = n
    for m in range(N_MM):
        if m == 8:
            n += 1
            pe_idx["T1"] = n
        n += 1
        pe_idx[m] = n

    # Store list: (rc, colbase, width, m_last).  (c0, rc0) split into
    # 4x 1024 cols; every other (c, rc) is 2x 2048 (one per half).
    stores = []
    for c in range(WCHUNKS):
        for rc in range(RCHUNKS):
            base_m = (c * RCHUNKS + rc) * 2 * SLICES
            for h in range(2):
                cb = h * HALF + c * WCW
                if c == 0 and rc == 0:
                    stores.append((rc, cb, 1024, base_m + h * SLICES + 1))
                    stores.append((rc, cb + 1024, 1024, base_m + h * SLICES + 3))
                else:
                    stores.append((rc, cb, WCW, base_m + h * SLICES + 3))

    ctx = ExitStack()
    with ctx:
        sb = lambda shape, tag, dt=f32: ctx.enter_context(  # noqa: E731
            nc.sbuf_tensor(tag, shape, dt)
        )
        aux_t = sb([128, A_COLS], "aux_t")
        iota_b = sb([128, E], "iota_b", bf16)
        w_t = sb([128, HALF], "w_t", bf16)
        # rc0 (DVE, bf16) and rc1 (GPSIMD, fp32) count/score chains
        eqs = [sb([128, E], f"eq{i}", bf16) for i in range(TOPK)]
        prs = [sb([128, E], f"pr{i}", bf16) for i in range(TOPK // 2)]
        qds = [sb([128, E], f"qd{i}", bf16) for i in range(TOPK // 4)]
        eqs2 = [sb([128, E], f"eqg{i}") for i in range(TOPK)]
        prs2 = [sb([128, E], f"prg{i}") for i in range(TOPK // 2)]
        qds2 = [sb([128, E], f"qdg{i}") for i in range(TOPK // 4)]
        cnt = [sb([128, E], f"cnt{r}") for r in range(RCHUNKS)]
        ct = [sb([128, 128], f"ct{r}", bf16) for r in range(RCHUNKS)]
        # staging: one [128, 4096] bf16 tensor per (c, rc) block
        stg = [sb([128, 2 * WCW], f"stg{g}", bf16) for g in range(WCHUNKS * RCHUNKS)]
        scr_a = sb([128, 128], "scr_a", bf16)
        scr_s = sb([128, D], "scr_s", bf16)

        ctp = [
            ctx.enter_context(nc.psum_tensor(f"ctp{r}", [E, 128], f32))
            for r in range(RCHUNKS)
        ]
        pmm = [
            ctx.enter_context(nc.psum_tensor(f"pmm{i}", [128, D], f32))
            for i in range(NPSUM)
        ]

        s_in = ctx.enter_context(nc.semaphore("s_in"))
        s_w = ctx.enter_context(nc.semaphore("s_w"))
        s_dve = ctx.enter_context(nc.semaphore("s_dve"))
        s_gp = ctx.enter_context(nc.semaphore("s_gp"))
        s_pe = ctx.enter_context(nc.semaphore("s_pe"))
        s_act = ctx.enter_context(nc.semaphore("s_act"))
        s_cpv = ctx.enter_context(nc.semaphore("s_cpv"))
        s_out = ctx.enter_context(nc.semaphore("s_out"))

        ident = lambda: aux_t[:, A_IDENT : A_IDENT + 128]  # noqa: E731
        iota_f = lambda: aux_t[:, A_IOTA : A_IOTA + E]  # noqa: E731
        sc = lambda r: aux_t[:, A_SC + r * E : A_SC + (r + 1) * E]  # noqa: E731
        idxcol = lambda r, k: aux_t[  # noqa: E731
            :, A_IDX + r * TOPK + k : A_IDX + r * TOPK + k + 1
        ]
        sgi = lambda c, rc: c * RCHUNKS + rc  # staging index  # noqa: E731

        block = ctx.enter_context(nc.Block())

        @block.sync
        def _(sp):
            sp.dma_start(out=aux_t[:], in_=aux_d[:]).then_inc(s_in, 16)
            for c in range(WCHUNKS):
                cols = slice(c * WCW, (c + 1) * WCW)
                sp.dma_start(out=w_t[:, cols], in_=wk_d[:, cols]).then_inc(s_w, 16)
            for rc, cb, width, m_last in stores:
                rows = slice(rc * 128, (rc + 1) * 128)
                thr = (m_last + 1) // 2  # copies alternate ACT(even)/DVE(odd)
                sp.wait_ge(s_act, thr)
                sp.wait_ge(s_cpv, thr)
                c = (cb % HALF) // WCW
                off = cb % WCW + (cb // HALF) * WCW
                sp.dma_start(
                    out=out_d[rows, cb : cb + width],
                    in_=stg[sgi(c, rc)][:, off : off + width],
                ).then_inc(s_out, 16)
            sp.wait_ge(s_out, 16 * len(stores))

        @block.vector
        def _(v):
            v.wait_ge(s_in, 16)
            v.tensor_copy(iota_b[:], iota_f())
            v.drain()
            for k in range(TOPK):
                v.tensor_scalar(
                    eqs[k][:], iota_b[:], idxcol(0, k), None,
                    mybir.AluOpType.is_equal,
                )
            v.drain()
            for i in range(TOPK // 2):
                v.tensor_add(prs[i][:], eqs[2 * i][:], eqs[2 * i + 1][:])
            v.drain()
            for i in range(TOPK // 4):
                v.tensor_add(qds[i][:], prs[2 * i][:], prs[2 * i + 1][:])
            v.drain()
            v.tensor_add(cnt[0][:], qds[0][:], qds[1][:])
            v.drain()
            v.tensor_mul(cnt[0][:], cnt[0][:], sc(0)).then_inc(s_dve, 1)
            v.wait_ge(s_pe, pe_idx["T0"])
            v.tensor_copy(ct[0][:E, :], ctp[0][:]).then_inc(s_dve, 1)
            v.tensor_copy(ct[0][E:, :], ctp[0][:]).then_inc(s_dve, 1)
            ct1_done = False
            for m, c, rc, h, s in mm_seq():
                if m % 2 == 0:
                    continue
                if m > 8 and not ct1_done:
                    v.wait_ge(s_pe, pe_idx["T1"])
                    v.tensor_copy(ct[1][:E, :], ctp[1][:]).then_inc(s_dve, 1)
                    v.tensor_copy(ct[1][E:, :], ctp[1][:]).then_inc(s_dve, 1)
                    ct1_done = True
                v.wait_ge(s_pe, pe_idx[m])
                gi = sgi(c, rc)
                v.tensor_copy(
                    stg[gi][:, (h * SLICES + s) * D : (h * SLICES + s + 1) * D],
                    pmm[m % NPSUM][:],
                ).then_inc(s_cpv, 1)

        @block.gpsimd
        def _(gp):
            gp.wait_ge(s_in, 16)
            for k in range(TOPK):
                gp.tensor_scalar(
                    eqs2[k][:], iota_f(), idxcol(1, k), None,
                    mybir.AluOpType.is_equal,
                )
            gp.drain()
            for i in range(TOPK // 2):
                gp.tensor_add(prs2[i][:], eqs2[2 * i][:], eqs2[2 * i + 1][:])
            gp.drain()
            for i in range(TOPK // 4):
                gp.tensor_add(qds2[i][:], prs2[2 * i][:], prs2[2 * i + 1][:])
            gp.drain()
            gp.tensor_add(cnt[1][:], qds2[0][:], qds2[1][:])
            gp.drain()
            gp.tensor_mul(cnt[1][:], cnt[1][:], sc(1)).then_inc(s_gp, 1)

        @block.tensor
        def _(t):
            for _ in range(N_WARM):
                t.matmul(
                    pmm[NPSUM - 1][:], scr_a[:E, :], scr_s[:E, :],
                    start=True, stop=True,
                )
            t.wait_ge(s_in, 16)  # ident (aux)
            t.wait_ge(s_dve, 1)
            t.transpose(ctp[0][:], cnt[0][:], ident()).then_inc(s_pe, 1)
            t.wait_ge(s_dve, 3)  # ct0 copies done
            cur_c = -1
            for m, c, rc, h, s in mm_seq():
                if m == 8:
                    t.wait_ge(s_gp, 1)
                    t.transpose(ctp[1][:], cnt[1][:], ident()).then_inc(s_pe, 1)
                    t.wait_ge(s_dve, 5)  # ct1 copies done
                if c != cur_c:
                    t.wait_ge(s_w, 16 * (c + 1))
                    cur_c = c
                if m >= NPSUM:
                    mm = m - NPSUM
                    if mm % 2 == 0:
                        t.wait_ge(s_act, mm // 2 + 1)
                    else:
                        t.wait_ge(s_cpv, mm // 2 + 1)
                pslice = slice(h * E, (h + 1) * E)
                wc = c * WCW + s * D
                t.matmul(
                    pmm[m % NPSUM][:],
                    ct[rc][pslice, :],
                    w_t[pslice, wc : wc + D],
                    start=True,
                    stop=True,
                ).then_inc(s_pe, 1)

        @block.scalar
        def _(a):
            for m, c, rc, h, s in mm_seq():
                if m % 2 == 1:
                    continue
                a.wait_ge(s_pe, pe_idx[m])
                gi = sgi(c, rc)
                a.copy(
                    stg[gi][:, (h * SLICES + s) * D : (h * SLICES + s + 1) * D],
                    pmm[m % NPSUM][:],
                ).then_inc(s_act, 1)

    return nc


def _run(selection_score, expert_indices, all_weight, trace=False):
    import ml_dtypes
    from concourse.bass_utils import run_bass_kernel_spmd

    bf16 = ml_dtypes.bfloat16
    scores = np.asarray(selection_score, dtype=np.float32)
    idxf = np.asarray(expert_indices).astype(np.float32)
    w = np.asarray(all_weight, dtype=np.float32).reshape(E, NF)
    wk = np.ascontiguousarray(
        w.reshape(E, 2, HALF).transpose(1, 0, 2).reshape(128, HALF).astype(bf16)
    )
    iota = np.tile(np.arange(E, dtype=np.float32), (128, 1))
    ident = np.eye(128, dtype=np.float32)

    if "nc" not in _cache:
        _cache["nc"] = _build_program()
    nc = _cache["nc"]

    in_maps = []
    for c in range(N_CORES):
        rows = slice(c * RPC, (c + 1) * RPC)
        sc = scores[rows].reshape(RCHUNKS, 128, E)
        ix = idxf[rows].reshape(RCHUNKS, 128, TOPK)
        aux = np.concatenate(
            [iota, ident, sc[0], sc[1], ix[0], ix[1]], axis=1, dtype=np.float32
        )
        in_maps.append({"aux": np.ascontiguousarray(aux), "wk": wk})
    r = run_bass_kernel_spmd(nc, in_maps, list(range(N_CORES)), trace=trace)
    full = np.concatenate(
        [np.asarray(r.results[c]["out"]).astype(np.float32) for c in range(N_CORES)],
        axis=0,
    )
    return full.reshape(BS, PL, D), r


def kernel(selection_score, expert_indices, all_weight) -> np.ndarray:
    full, _ = _run(selection_score, expert_indices, all_weight, trace=False)
    return full


# revision 15
# speedup vs baseline: 2.1307x; 1.0847x over previous
"""MoE routing mixture kernel for Trainium2 (8 NeuronCores, SPMD data-parallel).

Math: out[b] = sum_k selection_score[b, idx[b,k]] * all_weight[idx[b,k]]
Rewritten as a dense matmul: out = C @ W_flat, where
  C[b,e]    = selection_score[b,e] * |{k : idx[b,k]==e}|      ([2048, 64])
  W_flat    = all_weight.reshape(64, 16384)
Sharding: batch rows split across 8 cores (256 rows each); W replicated.

The timeline cost model serializes all DMA transfers on one DMA_ENGINES
resource at ~360 B/ns, so makespan ~= bytes moved / 360 + issue/sem
overheads.  W is therefore loaded and the output stored in bf16, halving
the dominant traffic (20.5 MiB -> ~10 MiB per core); matmuls run in bf16
(1 PE cycle/row vs fp32's 4).  End-to-end rounding error ~4e-3 rel.

Raw Bass (no Tile): descriptors carry at most one sync wait and one sync
update each, so synchronization is standalone wait_ge instructions plus
.then_inc updates, one per instruction.

Head-latency optimizations (the store phase is DMA-back-to-back; the
makespan is set by when the first store's data is ready):
  - aux DMA issued from ACT, whose block preamble is ~700ns shorter than
    SP's, so the input arrives earlier; W + stores issue from SP.
  - GPSIMD computes row-chunk 1's C = score*count chain in parallel with
    DVE's row-chunk 0.
  - One PE warmup matmul on scratch: any later sem-gated PE instruction
    (resolving after the ~3us p-state ramp) then runs at full clock.
  - PE order: T0, c0/rc0 matmuls, T1, rest -- transpose1 (which needs
    GPSIMD's C1) doesn't block the first store group.
  - C^T PSUM->SBUF copies split DVE/ACT (run in parallel).
  - All (c0, *) groups stored as 1024-col DMAs so stores start earlier.

Pipeline per core (256 rows = 2 row chunks of 128):
  SP   : 4 W-chunk DMAs (bf16) -> 20 output DMAs (bf16)
  ACT  : aux DMA; ct[r] upper-half copies; even PSUM->SBUF copies
  DVE  : rc0 C chain (bf16 eq/add tree); ct[r] lower-half copies; odd
         PSUM->SBUF copies
  Pool : rc1 C chain (fp32)
  PE   : warmup, 2 transposes, 64 bf16 matmuls [64x128]@[64x512]
"""

import sys
from contextlib import ExitStack

import numpy as np

sys.path.insert(0, "/opt/trn_rl_repo")

BS, E, TOPK, PL, D = 2048, 64, 8, 32, 512
NF = PL * D  # 16384 flattened prompt*dim
N_CORES = 8
RPC = BS // N_CORES  # 256 rows per core
RCHUNKS = RPC // 128  # 2 row chunks of 128
HALF = NF // 2  # 8192: W stored on-chip as [128, 8192] bf16
WCHUNKS = 4  # W loaded in 4 chunks of [128, 2048]
WCW = HALF // WCHUNKS  # 2048
SLICES = WCW // D  # 4 matmuls (512 cols) per (chunk, rowchunk, half)
NPSUM = 7  # matmul PSUM ring

# aux tensor column layout (fp32): iota | ident | sc0 | sc1 | idx0 | idx1
A_IOTA = 0
A_IDENT = A_IOTA + E  # 64
A_SC = A_IDENT + 128  # 192
A_IDX = A_SC + RCHUNKS * E  # 320 (idx stored as fp32 = 2 bf16 cols each)
A_COLS = A_IDX + RCHUNKS * TOPK * 2  # 352

_cache: dict = {}


def _build_program():
    import concourse.bass as bass
    import concourse.mybir as mybir

    f32 = mybir.dt.float32
    bf16 = mybir.dt.bfloat16
    nc = bass.Bass()

    aux_d = nc.declare_dram_parameter("aux", [128, A_COLS], bf16, isOutputFalse=False) if False else nc.declare_dram_parameter("aux", [128, A_COLS], bf16, isOutput=False)
    # W_flat [64, 16384] bf16 on partitions 0:64, columns c-major:
    # col c*4096 + h*2048 + s*512 holds output cols h*8192 + c*2048 + s*512.
    wk_d = nc.declare_dram_parameter("wk", [64, NF], bf16, isOutput=False)
    out_d = nc.declare_dram_parameter("out", [RPC, NF], bf16, isOutput=True)

    # matmul m (PE order) -> (wchunk c, rowchunk rc, half h, slice s)
    def mm_seq():
        m = 0
        for c in range(WCHUNKS):
            for rc in range(RCHUNKS):
                for h in range(2):
                    for s in range(SLICES):
                        yield m, c, rc, h, s
                        m += 1

    N_MM = WCHUNKS * RCHUNKS * 2 * SLICES  # 64

    # s_pe increment index of each matmul / transpose (warmup doesn't inc;
    # T1 sits between m=7 and m=8).
    pe_idx: dict = {}
    n = 1
    pe_idx["T0"] = n
    for m in range(N_MM):
        if m == 8:
            n += 1
            pe_idx["T1"] = n
        n += 1
        pe_idx[m] = n

    # Store list: (rc, colbase, width, m_last).  c0 groups split into
    # 1024-col stores; every other (c, rc, h) is one 2048-col store.
    stores = []
    for c in range(WCHUNKS):
        for rc in range(RCHUNKS):
            base_m = (c * RCHUNKS + rc) * 2 * SLICES
            for h in range(2):
                cb = h * HALF + c * WCW
                if c == 0:
                    stores.append((rc, cb, 1024, base_m + h * SLICES + 1))
                    stores.append((rc, cb + 1024, 1024, base_m + h * SLICES + 3))
                else:
                    stores.append((rc, cb, WCW, base_m + h * SLICES + 3))

    ctx = ExitStack()
    with ctx:
        sb = lambda shape, tag, dt=f32: ctx.enter_context(  # noqa: E731
            nc.sbuf_tensor(tag, shape, dt)
        )
        aux_t = sb([128, A_COLS], "aux_t", bf16)
        w_t = sb([64, NF], "w_t", bf16)
        # rc0 (DVE, bf16) and rc1 (GPSIMD, fp32) count/score chains
        eqs = [sb([128, E], f"eq{i}", bf16) for i in range(TOPK)]
        prs = [sb([128, E], f"pr{i}", bf16) for i in range(TOPK // 2)]
        qds = [sb([128, E], f"qd{i}", bf16) for i in range(TOPK // 4)]
        eqs2 = [sb([128, E], f"eqg{i}", bf16) for i in range(TOPK)]
        prs2 = [sb([128, E], f"prg{i}", bf16) for i in range(TOPK // 2)]
        qds2 = [sb([128, E], f"qdg{i}", bf16) for i in range(TOPK // 4)]
        cnt = [sb([128, E], f"cnt{r}", bf16) for r in range(RCHUNKS)]
        ct = [sb([64, 128], f"ct{r}", bf16) for r in range(RCHUNKS)]
        # staging: one [128, 4096] bf16 tensor per (c, rc) block
        stg = [sb([128, 2 * WCW], f"stg{g}", bf16) for g in range(WCHUNKS * RCHUNKS)]
        scr_a = sb([128, 128], "scr_a", bf16)
        scr_s = sb([128, D], "scr_s", bf16)

        ctp_all = ctx.enter_context(nc.psum_tensor("ctp", [E, 256], bf16))
        ctp = [ctp_all[:, r * 128 : (r + 1) * 128] for r in range(RCHUNKS)]
        pmm = [
            ctx.enter_context(nc.psum_tensor(f"pmm{i}", [128, D], f32))
            for i in range(NPSUM)
        ]

        s_in = ctx.enter_context(nc.semaphore("s_in"))
        s_w = [
            ctx.enter_context(nc.semaphore(f"s_w{c}")) for c in range(WCHUNKS)
        ]
        s_dve = ctx.enter_context(nc.semaphore("s_dve"))
        s_gp = ctx.enter_context(nc.semaphore("s_gp"))
        s_ct = ctx.enter_context(nc.semaphore("s_ct"))
        s_pe = ctx.enter_context(nc.semaphore("s_pe"))
        s_act = ctx.enter_context(nc.semaphore("s_act"))
        s_cpv = ctx.enter_context(nc.semaphore("s_cpv"))
        s_out = ctx.enter_context(nc.semaphore("s_out"))

        ident = lambda: aux_t[:, A_IDENT : A_IDENT + 128]  # noqa: E731
        iota_f = lambda: aux_t[:, A_IOTA : A_IOTA + E]  # noqa: E731
        sc = lambda r: aux_t[:, A_SC + r * E : A_SC + (r + 1) * E]  # noqa: E731
        idxcol = lambda r, k: aux_t[  # noqa: E731
            :, A_IDX + 2 * (r * TOPK + k) : A_IDX + 2 * (r * TOPK + k) + 2
        ].bitcast(f32)
        sgi = lambda c, rc: c * RCHUNKS + rc  # staging index  # noqa: E731
        stg_sl = lambda c, rc, h, s: stg[sgi(c, rc)][  # noqa: E731
            :, (h * SLICES + s) * D : (h * SLICES + s + 1) * D
        ]

        block = ctx.enter_context(nc.Block())

        @block.sync
        def _(sp):
            sp.dma_start(out=aux_t[:], in_=aux_d[:]).then_inc(s_in, 16)
            for c in range(WCHUNKS):
                cols = slice(c * 2 * WCW, (c + 1) * 2 * WCW)
                sp.dma_start(out=w_t[:, cols], in_=wk_d[:, cols]).then_inc(
                    s_w[c], 16
                )
            for rc, cb, width, m_last in stores:
                rows = slice(rc * 128, (rc + 1) * 128)
                thr = (m_last + 1) // 2  # copies alternate ACT(even)/DVE(odd)
                sp.wait_ge(s_act, thr)
                sp.wait_ge(s_cpv, thr)
                c = (cb % HALF) // WCW
                off = cb % WCW + (cb // HALF) * WCW
                sp.dma_start(
                    out=out_d[rows, cb : cb + width],
                    in_=stg[sgi(c, rc)][:, off : off + width],
                ).then_inc(s_out, 16)
            sp.wait_ge(s_out, 16 * len(stores))

        @block.vector
        def _(v):
            v.wait_ge(s_in, 16)
            for k in range(TOPK):
                v.tensor_scalar(
                    eqs[k][:], iota_f(), idxcol(0, k), None,
                    mybir.AluOpType.is_equal,
                )
            v.drain()
            for i in range(TOPK // 2):
                v.tensor_add(prs[i][:], eqs[2 * i][:], eqs[2 * i + 1][:])
            v.drain()
            for i in range(TOPK // 4):
                v.tensor_add(qds[i][:], prs[2 * i][:], prs[2 * i + 1][:])
            v.drain()
            v.tensor_add(cnt[0][:], qds[0][:], qds[1][:])
            v.drain()
            v.tensor_mul(cnt[0][:], cnt[0][:], sc(0)).then_inc(s_dve, 1)
            v.wait_ge(s_pe, pe_idx["T0"])
            v.tensor_copy(ct[0][:], ctp[0][:]).then_inc(s_ct, 1)
            ct1_done = False
            for m, c, rc, h, s in mm_seq():
                if m % 2 == 0:
                    continue
                if m > 5 and not ct1_done:
                    v.wait_ge(s_pe, pe_idx["T1"])
                    v.tensor_copy(ct[1][:], ctp[1][:]).then_inc(s_ct, 1)
                    ct1_done = True
                v.wait_ge(s_pe, pe_idx[m])
                v.tensor_copy(stg_sl(c, rc, h, s), pmm[m % NPSUM][:]).then_inc(
                    s_cpv, 1
                )

        @block.gpsimd
        def _(gp):
            gp.memset(scr_a[:], 0)
            gp.memset(scr_s[:], 0).then_inc(s_gp, 1)
            gp.wait_ge(s_in, 16)
            for k in range(TOPK):
                gp.tensor_scalar(
                    eqs2[k][:], iota_f(), idxcol(1, k), None,
                    mybir.AluOpType.is_equal,
                )
            gp.drain()
            for i in range(TOPK // 2):
                gp.tensor_add(prs2[i][:], eqs2[2 * i][:], eqs2[2 * i + 1][:])
            gp.drain()
            for i in range(TOPK // 4):
                gp.tensor_add(qds2[i][:], prs2[2 * i][:], prs2[2 * i + 1][:])
            gp.drain()
            gp.tensor_add(cnt[1][:], qds2[0][:], qds2[1][:])
            gp.drain()
            gp.tensor_mul(cnt[1][:], cnt[1][:], sc(1)).then_inc(s_gp, 1)  # -> 2

        @block.tensor
        def _(t):
            t.wait_ge(s_gp, 1)  # scratch memset done
            t.matmul(
                pmm[NPSUM - 1][:], scr_a[:E, :], scr_s[:E, :],
                start=True, stop=True,
            )
            t.wait_ge(s_in, 16)  # ident (aux); also paces the p-state ramp:
            # idle gaps > 3us reset pe_busy_start, so a second warmup here
            # (~3.7us) keeps every later gap under 3us and the real matmuls
            # at full clock.
            t.matmul(
                pmm[NPSUM - 1][:], scr_a[:E, :], scr_s[:E, :],
                start=True, stop=True,
            )
            t.wait_ge(s_dve, 1)
            t.transpose(ctp[0][:], cnt[0][:], ident()).then_inc(s_pe, 1)
            t.wait_ge(s_ct, 1)  # ct0 copy done
            cur_c = -1
            for m, c, rc, h, s in mm_seq():
                if m == 8:
                    t.wait_ge(s_gp, 2)
                    t.transpose(ctp[1][:], cnt[1][:], ident()).then_inc(s_pe, 1)
                    t.wait_ge(s_ct, 2)  # ct1 copy done
                if c != cur_c:
                    t.wait_ge(s_w[c], 16)
                    cur_c = c
                if m >= NPSUM:
                    mm = m - NPSUM
                    if mm % 2 == 0:
                        t.wait_ge(s_act, mm // 2 + 1)
                    else:
                        t.wait_ge(s_cpv, mm // 2 + 1)
                wc = c * 2 * WCW + h * WCW + s * D
                t.matmul(
                    pmm[m % NPSUM][:],
                    ct[rc][:],
                    w_t[:, wc : wc + D],
                    start=True,
                    stop=True,
                ).then_inc(s_pe, 1)

        @block.scalar
        def _(a):
            for m, c, rc, h, s in mm_seq():
                if m % 2 == 1:
                    continue
                a.wait_ge(s_pe, pe_idx[m])
                a.copy(stg_sl(c, rc, h, s), pmm[m % NPSUM][:]).then_inc(s_act, 1)

    return nc


def _run(selection_score, expert_indices, all_weight, trace=False):
    import ml_dtypes
    from concourse.bass_utils import run_bass_kernel_spmd

    bf16 = ml_dtypes.bfloat16
    scores = np.asarray(selection_score, dtype=np.float32)
    idxf = np.asarray(expert_indices).astype(np.float32)
    w = np.asarray(all_weight, dtype=np.float32).reshape(E, NF)
    # [e, h, c, 2048] -> [e, c, h, 2048] so each W-chunk DMA is contiguous
    wk = np.ascontiguousarray(
        w.reshape(E, 2, WCHUNKS, WCW).transpose(0, 2, 1, 3).reshape(E, NF).astype(bf16)
    )
    iota = np.tile(np.arange(E, dtype=np.float32), (128, 1))
    ident = np.eye(128, dtype=np.float32)

    if "nc" not in _cache:
        _cache["nc"] = _build_program()
    nc = _cache["nc"]

    in_maps = []
    for c in range(N_CORES):
        rows = slice(c * RPC, (c + 1) * RPC)
        scb = scores[rows].reshape(RCHUNKS, 128, E)
        ix = idxf[rows].reshape(RCHUNKS, 128, TOPK)
        aux = np.concatenate(
            [iota, ident, scb[0], scb[1]], axis=1, dtype=np.float32
        ).astype(bf16)
        # idx values stay fp32, byte-spliced into the bf16 tensor (device
        # bitcasts the 2-col pairs back to fp32 scalars)
        idx_bytes = np.concatenate([ix[0], ix[1]], axis=1, dtype=np.float32)
        aux = np.concatenate([aux, idx_bytes.view(bf16)], axis=1)
        in_maps.append({"aux": np.ascontiguousarray(aux), "wk": wk})
    r = run_bass_kernel_spmd(nc, in_maps, list(range(N_CORES)), trace=trace)
    full = np.concatenate(
        [np.asarray(r.results[c]["out"]).astype(np.float32) for c in range(N_CORES)],
        axis=0,
    )
    return full.reshape(BS, PL, D), r


def kernel(selection_score, expert_indices, all_weight) -> np.ndarray:
    full, _ = _run(selection_score, expert_indices, all_weight, trace=False)
    return full


# revision 25
# speedup vs baseline: 2.1781x; 1.0222x over previous
"""MoE routing mixture kernel for Trainium2 (8 NeuronCores, SPMD data-parallel).

Math: out[b] = sum_k selection_score[b, idx[b,k]] * all_weight[idx[b,k]]
Rewritten as a dense matmul: out = C @ W_flat, where
  C[b,e]    = selection_score[b,e] * |{k : idx[b,k]==e}|      ([2048, 64])
  W_flat    = all_weight.reshape(64, 16384)
Sharding: batch rows split across 8 cores (256 rows each); W replicated.

The timeline cost model serializes all DMA transfers on one DMA_ENGINES
resource at ~360 B/ns, so makespan ~= bytes moved / 360 + issue/sem
overheads.  W is loaded and the output stored in bf16, halving the
dominant traffic (20.5 MiB -> ~10 MiB per core); matmuls run in bf16 at
1 PE cycle/row and write bf16 straight to PSUM, so the PSUM->SBUF
staging copies move 2-byte data (DVE gets its 2x mode).  End-to-end
rounding error ~3e-3 rel vs the 2e-2 gate.

Raw Bass (no Tile): descriptors carry at most one sync wait and one sync
update each, so synchronization is standalone wait_ge instructions plus
.then_inc updates, one per instruction.  Same-engine RAW chains on DVE
need explicit drain()s (engine writeback is pipelined).

Head-latency design (the store phase is DMA-back-to-back; makespan is
set by when the first store's data is ready, relative to the fixed end
of the W-load phase):
  - One fused bf16 aux DMA (iota|ident|scores|idx-as-fp32-bytes) issued
    first from SP; idx scalars are bitcast back to fp32 slices on chip.
  - GPSIMD computes row-chunk 1's C chain in parallel with DVE's rc0.
  - Two spaced PE warmup matmuls on zeroed scratch keep every PE idle
    gap under the 3us p-state reset, so real matmuls run at full clock.
  - PE order: T0, c0/rc0 matmuls, T1, rest -- T1 (which needs GPSIMD's
    C1) doesn't block the first store group.
  - Stores alternate DVE/ACT ownership: both staging copies of a store
    come from one engine, so each store DMA needs a single sem wait and
    SP can issue stores faster than the DMA transfers them.
  - c0 store groups are 1024 cols (2 copies); later groups 2048 (4).

Pipeline per core (256 rows = 2 row chunks of 128):
  SP   : aux DMA -> 4 W-chunk DMAs (bf16) -> 20 output DMAs (bf16)
  DVE  : rc0 C chain (bf16 eq/add tree); ct0 copy; even-store copies
  ACT  : ct1 copy; odd-store copies
  Pool : scratch memsets; rc1 C chain (bf16)
  PE   : warmups, 2 transposes, 64 bf16 matmuls [64x128]@[64x512]
"""

import sys
from contextlib import ExitStack

import numpy as np

sys.path.insert(0, "/opt/trn_rl_repo")

BS, E, TOPK, PL, D = 2048, 64, 8, 32, 512
NF = PL * D  # 16384 flattened prompt*dim
N_CORES = 8
RPC = BS // N_CORES  # 256 rows per core
RCHUNKS = RPC // 128  # 2 row chunks of 128
HALF = NF // 2  # 8192 output cols per half
WCHUNKS = 4  # W loaded in 4 chunks of [64, 4096]
WCW = HALF // WCHUNKS  # 2048
SLICES = WCW // D  # 4 matmuls (512 cols) per (chunk, rowchunk, half)
NPSUM = 7  # matmul PSUM ring (one PSUM bank each; ctp uses the 8th)

# aux tensor column layout (bf16): iota | ident | sc0 | sc1 | idx (fp32 bytes)
A_IOTA = 0
A_IDENT = A_IOTA + E  # 64
A_SC = A_IDENT + 128  # 192
A_IDX = A_SC + RCHUNKS * E  # 320 (idx stored as fp32 = 2 bf16 cols each)
A_COLS = A_IDX + RCHUNKS * TOPK * 2  # 352

_cache: dict = {}


def _build_program():
    import concourse.bass as bass
    import concourse.mybir as mybir

    f32 = mybir.dt.float32
    bf16 = mybir.dt.bfloat16
    nc = bass.Bass()

    aux_d = nc.declare_dram_parameter("aux", [128, A_COLS], bf16, isOutput=False)
    # W_flat [64, 16384] bf16 on partitions 0:64, columns c-major:
    # col c*4096 + h*2048 + s*512 holds output cols h*8192 + c*2048 + s*512.
    wk_d = nc.declare_dram_parameter("wk", [64, NF], bf16, isOutput=False)
    out_d = nc.declare_dram_parameter("out", [RPC, NF], bf16, isOutput=True)

    # matmul m (PE order) -> (wchunk c, rowchunk rc, half h, slice s)
    def mm_seq():
        m = 0
        for c in range(WCHUNKS):
            for rc in range(RCHUNKS):
                for h in range(2):
                    for s in range(SLICES):
                        yield m, c, rc, h, s
                        m += 1

    N_MM = WCHUNKS * RCHUNKS * 2 * SLICES  # 64
    mm_info = {m: (c, rc, h, s) for m, c, rc, h, s in mm_seq()}

    # s_pe increment index of each matmul / transpose (warmups don't inc;
    # T1 sits between m=7 and m=8).
    pe_idx: dict = {}
    n = 1
    pe_idx["T0"] = n
    for m in range(N_MM):
        if m == 8:
            n += 1
            pe_idx["T1"] = n
        n += 1
        pe_idx[m] = n

    # Store list: (rc, colbase, width, [matmul indices]).  c0 groups are
    # 1024-col stores (2 slices); every other (c, rc, h) one 2048-col store.
    stores = []
    for c in range(WCHUNKS):
        for rc in range(RCHUNKS):
            base_m = (c * RCHUNKS + rc) * 2 * SLICES
            for h in range(2):
                cb = h * HALF + c * WCW
                m0 = base_m + h * SLICES
                stores.append((rc, cb, 1024, [m0, m0 + 1]))
                stores.append((rc, cb + 1024, 1024, [m0 + 2, m0 + 3]))

    # Copy assignment: slices alternate engines by matmul parity (even m ->
    # ACT, odd m -> DVE) so both engines advance every store.  copy_pos[m] =
    # (eng, 1-based position in that engine's copy stream); store_thr[g] =
    # per-engine wait thresholds for store g.
    copy_pos: dict = {}
    eng_seq: dict = {"v": [], "a": []}
    for g, (rc, cb, width, ms) in enumerate(stores):
        for m in ms:
            eng = "a" if m % 2 == 0 else "v"
            eng_seq[eng].append(m)
            copy_pos[m] = (eng, len(eng_seq[eng]))
    store_thr = []
    for rc, cb, width, ms in stores:
        thr: dict = {}
        for m in ms:
            eng, pos = copy_pos[m]
            thr[eng] = max(thr.get(eng, 0), pos)
        store_thr.append(thr)

    ctx = ExitStack()
    with ctx:
        sb = lambda shape, tag, dt=bf16: ctx.enter_context(  # noqa: E731
            nc.sbuf_tensor(tag, shape, dt)
        )
        aux_t = sb([128, A_COLS], "aux_t")
        w_t = sb([64, NF], "w_t")
        # rc0 (DVE) and rc1 (GPSIMD) count/score chains, all bf16
        eqs = [sb([128, E], f"eq{i}") for i in range(TOPK)]
        prs = [sb([128, E], f"pr{i}") for i in range(TOPK // 2)]
        qds = [sb([128, E], f"qd{i}") for i in range(TOPK // 4)]
        eqs2 = [sb([128, E], f"eqg{i}") for i in range(TOPK)]
        prs2 = [sb([128, E], f"prg{i}") for i in range(TOPK // 2)]
        qds2 = [sb([128, E], f"qdg{i}") for i in range(TOPK // 4)]
        cnt = [sb([128, E], f"cnt{r}") for r in range(RCHUNKS)]
        ct = [sb([64, 128], f"ct{r}") for r in range(RCHUNKS)]
        # staging: one [128, 4096] bf16 tensor per (c, rc) block
        stg = [sb([128, 2 * WCW], f"stg{g}") for g in range(WCHUNKS * RCHUNKS)]
        scr_a = sb([128, 128], "scr_a")
        scr_s = sb([128, D], "scr_s")

        ctp_all = ctx.enter_context(nc.psum_tensor("ctp", [E, 256], bf16))
        ctp = [ctp_all[:, r * 128 : (r + 1) * 128] for r in range(RCHUNKS)]
        pmm = [
            ctx.enter_context(nc.psum_tensor(f"pmm{i}", [128, D], f32))
            for i in range(NPSUM)
        ]

        s_in = ctx.enter_context(nc.semaphore("s_in"))
        s_w = [
            ctx.enter_context(nc.semaphore(f"s_w{c}")) for c in range(WCHUNKS)
        ]
        s_dve = ctx.enter_context(nc.semaphore("s_dve"))
        s_gp = ctx.enter_context(nc.semaphore("s_gp"))
        s_ct0 = ctx.enter_context(nc.semaphore("s_ct0"))
        s_ct1 = ctx.enter_context(nc.semaphore("s_ct1"))
        s_pe = ctx.enter_context(nc.semaphore("s_pe"))
        s_act = ctx.enter_context(nc.semaphore("s_act"))
        s_cpv = ctx.enter_context(nc.semaphore("s_cpv"))
        s_out = ctx.enter_context(nc.semaphore("s_out"))
        sem_of = {"v": s_cpv, "a": s_act}

        ident = lambda: aux_t[:, A_IDENT : A_IDENT + 128]  # noqa: E731
        iota_f = lambda: aux_t[:, A_IOTA : A_IOTA + E]  # noqa: E731
        sc = lambda r: aux_t[:, A_SC + r * E : A_SC + (r + 1) * E]  # noqa: E731
        idxcol = lambda r, k: aux_t[  # noqa: E731
            :, A_IDX + 2 * (r * TOPK + k) : A_IDX + 2 * (r * TOPK + k) + 2
        ].bitcast(f32)
        sgi = lambda c, rc: c * RCHUNKS + rc  # staging index  # noqa: E731

        def stg_sl(m):
            c, rc, h, s = mm_info[m]
            col = (h * SLICES + s) * D
            return stg[sgi(c, rc)][:, col : col + D]

        block = ctx.enter_context(nc.Block())

        @block.sync
        def _(sp):
            sp.dma_start(out=aux_t[:], in_=aux_d[:]).then_inc(s_in, 16)
            for c in range(WCHUNKS):
                cols = slice(c * 2 * WCW, (c + 1) * 2 * WCW)
                sp.dma_start(out=w_t[:, cols], in_=wk_d[:, cols]).then_inc(
                    s_w[c], 16
                )
            for g, (rc, cb, width, ms) in enumerate(stores):
                rows = slice(rc * 128, (rc + 1) * 128)
                c = (cb % HALF) // WCW
                off = cb % WCW + (cb // HALF) * WCW
                waits = list(store_thr[g].items())
                for eng, pos in waits[:-1]:
                    sp.wait_ge(sem_of[eng], pos)
                # last wait rides on the DMA descriptor itself (one sync
                # wait per instruction), saving a standalone wait per store
                sp.dma_start(
                    out=out_d[rows, cb : cb + width],
                    in_=stg[sgi(c, rc)][:, off : off + width],
                )._wait_ge(sem_of[waits[-1][0]], waits[-1][1]).then_inc(s_out, 16)

        @block.vector
        def _(v):
            v.wait_ge(s_in, 16)
            for k in range(TOPK):
                v.tensor_scalar(
                    eqs[k][:], iota_f(), idxcol(0, k), None,
                    mybir.AluOpType.is_equal,
                )
            v.drain()
            for i in range(TOPK // 2):
                v.tensor_add(prs[i][:], eqs[2 * i][:], eqs[2 * i + 1][:])
            v.drain()
            for i in range(TOPK // 4):
                v.tensor_add(qds[i][:], prs[2 * i][:], prs[2 * i + 1][:])
            v.drain()
            v.tensor_add(cnt[0][:], qds[0][:], qds[1][:])
            v.drain()
            v.tensor_mul(cnt[0][:], cnt[0][:], sc(0)).then_inc(s_dve, 1)
            v.wait_ge(s_pe, pe_idx["T0"])
            v.tensor_copy(ct[0][:], ctp[0][:]).then_inc(s_ct0, 1)
            ct1_done = False
            for m in eng_seq["v"]:
                if m > 5 and not ct1_done:
                    v.wait_ge(s_pe, pe_idx["T1"])
                    v.tensor_copy(ct[1][:], ctp[1][:]).then_inc(s_ct1, 1)
                    ct1_done = True
                v.wait_ge(s_pe, pe_idx[m])
                v.tensor_copy(stg_sl(m), pmm[m % NPSUM][:]).then_inc(s_cpv, 1)

        @block.scalar
        def _(a):
            for m in eng_seq["a"]:
                a.wait_ge(s_pe, pe_idx[m])
                a.copy(stg_sl(m), pmm[m % NPSUM][:]).then_inc(s_act, 1)

        @block.gpsimd
        def _(gp):
            gp.memset(scr_a[:], 0)
            gp.memset(scr_s[:], 0).then_inc(s_gp, 1)
            gp.wait_ge(s_in, 16)
            for k in range(TOPK):
                gp.tensor_scalar(
                    eqs2[k][:], iota_f(), idxcol(1, k), None,
                    mybir.AluOpType.is_equal,
                )
            gp.drain()
            for i in range(TOPK // 2):
                gp.tensor_add(prs2[i][:], eqs2[2 * i][:], eqs2[2 * i + 1][:])
            gp.drain()
            for i in range(TOPK // 4):
                gp.tensor_add(qds2[i][:], prs2[2 * i][:], prs2[2 * i + 1][:])
            gp.drain()
            gp.tensor_add(cnt[1][:], qds2[0][:], qds2[1][:])
            gp.drain()
            gp.tensor_mul(cnt[1][:], cnt[1][:], sc(1)).then_inc(s_gp, 1)  # -> 2

        @block.tensor
        def _(t):
            t.wait_ge(s_gp, 1)  # scratch memset done
            t.matmul(
                pmm[NPSUM - 1][:], scr_a[:E, :], scr_s[:E, :],
                start=True, stop=True,
            )
            t.wait_ge(s_in, 16)  # ident (aux); also paces the p-state ramp:
            # idle gaps > 3us reset pe_busy_start, so a second warmup here
            # (~3.4us) keeps every later gap under 3us and the real matmuls
            # at full clock.
            t.matmul(
                pmm[NPSUM - 1][:], scr_a[:E, :], scr_s[:E, :],
                start=True, stop=True,
            )
            t.wait_ge(s_dve, 1)
            t.transpose(ctp[0][:], cnt[0][:], ident()).then_inc(s_pe, 1)
            t.wait_ge(s_ct0, 1)  # ct0 copy done
            cur_c = -1
            for m, c, rc, h, s in mm_seq():
                if m == 8:
                    t.wait_ge(s_gp, 2)
                    t.transpose(ctp[1][:], cnt[1][:], ident()).then_inc(s_pe, 1)
                    t.wait_ge(s_ct1, 1)  # ct1 copy done
                if c != cur_c:
                    t.wait_ge(s_w[c], 16)
                    cur_c = c
                if m >= NPSUM:
                    eng, pos = copy_pos[m - NPSUM]
                    t.wait_ge(sem_of[eng], pos)
                wc = c * 2 * WCW + h * WCW + s * D
                t.matmul(
                    pmm[m % NPSUM][:],
                    ct[rc][:],
                    w_t[:, wc : wc + D],
                    start=True,
                    stop=True,
                ).then_inc(s_pe, 1)

    return nc


def _run(selection_score, expert_indices, all_weight, trace=False):
    import ml_dtypes
    from concourse.bass_utils import run_bass_kernel_spmd

    bf16 = ml_dtypes.bfloat16
    scores = np.asarray(selection_score, dtype=np.float32)
    idxf = np.asarray(expert_indices).astype(np.float32)
    w = np.asarray(all_weight, dtype=np.float32).reshape(E, NF)
    # [e, h, c, 2048] -> [e, c, h, 2048] so each W-chunk DMA is contiguous
    wk = np.ascontiguousarray(
        w.reshape(E, 2, WCHUNKS, WCW).transpose(0, 2, 1, 3).reshape(E, NF).astype(bf16)
    )
    iota = np.tile(np.arange(E, dtype=np.float32), (128, 1))
    ident = np.eye(128, dtype=np.float32)

    if "nc" not in _cache:
        _cache["nc"] = _build_program()
    nc = _cache["nc"]

    in_maps = []
    for c in range(N_CORES):
        rows = slice(c * RPC, (c + 1) * RPC)
        scb = scores[rows].reshape(RCHUNKS, 128, E)
        ix = idxf[rows].reshape(RCHUNKS, 128, TOPK)
        aux = np.concatenate(
            [iota, ident, scb[0], scb[1]], axis=1, dtype=np.float32
        ).astype(bf16)
        # idx values stay fp32, byte-spliced into the bf16 tensor (device
        # bitcasts the 2-col pairs back to fp32 scalars)
        idx_bytes = np.concatenate([ix[0], ix[1]], axis=1, dtype=np.float32)
        aux = np.concatenate([aux, idx_bytes.view(bf16)], axis=1)
        in_maps.append({"aux": np.ascontiguousarray(aux), "wk": wk})
    r = run_bass_kernel_spmd(nc, in_maps, list(range(N_CORES)), trace=trace)
    full = np.concatenate(
        [np.asarray(r.results[c]["out"]).astype(np.float32) for c in range(N_CORES)],
        axis=0,
    )
    return full.reshape(BS, PL, D), r


def kernel(selection_score, expert_indices, all_weight) -> np.ndarray:
    full, _ = _run(selection_score, expert_indices, all_weight, trace=False)
    return full


# revision 32
# speedup vs baseline: 2.1837x; 1.0026x over previous
"""MoE routing mixture kernel for Trainium2 (8 NeuronCores, SPMD data-parallel).

Math: out[b] = sum_k selection_score[b, idx[b,k]] * all_weight[idx[b,k]]
Rewritten as a dense matmul: out = C @ W_flat, where
  C[b,e]    = selection_score[b,e] * |{k : idx[b,k]==e}|      ([2048, 64])
  W_flat    = all_weight.reshape(64, 16384)
Sharding: batch rows split across 8 cores (256 rows each); W replicated.

The timeline cost model serializes all DMA transfers on one DMA_ENGINES
resource at ~360 B/ns, so makespan ~= bytes moved / 360 + issue/sem
overheads.  W is loaded and the output stored in bf16, halving the
dominant traffic (20.5 MiB -> ~10 MiB per core); matmuls run in bf16 at
1 PE cycle/row and write bf16 straight to PSUM, so the PSUM->SBUF
staging copies move 2-byte data (DVE gets its 2x mode).  End-to-end
rounding error ~3e-3 rel vs the 2e-2 gate.

Raw Bass (no Tile): descriptors carry at most one sync wait and one sync
update each, so synchronization is standalone wait_ge instructions plus
.then_inc updates, one per instruction.  Same-engine RAW chains on DVE
need explicit drain()s (engine writeback is pipelined).

Head-latency design (the store phase is DMA-back-to-back; makespan is
set by when the first store's data is ready, relative to the fixed end
of the W-load phase):
  - One fused bf16 aux DMA (iota|ident|scores|idx-as-fp32-bytes) issued
    first from SP; idx scalars are bitcast back to fp32 slices on chip.
  - GPSIMD computes row-chunk 1's C chain in parallel with DVE's rc0.
  - Two spaced PE warmup matmuls on zeroed scratch keep every PE idle
    gap under the 3us p-state reset, so real matmuls run at full clock.
  - PE order: T0, c0/rc0 matmuls, T1, rest -- T1 (which needs GPSIMD's
    C1) doesn't block the first store group.
  - Stores alternate DVE/ACT ownership: both staging copies of a store
    come from one engine, so each store DMA needs a single sem wait and
    SP can issue stores faster than the DMA transfers them.
  - c0 store groups are 1024 cols (2 copies); later groups 2048 (4).

Pipeline per core (256 rows = 2 row chunks of 128):
  SP   : aux DMA -> 4 W-chunk DMAs (bf16) -> 20 output DMAs (bf16)
  DVE  : rc0 C chain (bf16 eq/add tree); ct0 copy; even-store copies
  ACT  : ct1 copy; odd-store copies
  Pool : scratch memsets; rc1 C chain (bf16)
  PE   : warmups, 2 transposes, 64 bf16 matmuls [64x128]@[64x512]
"""

import sys
from contextlib import ExitStack

import numpy as np

sys.path.insert(0, "/opt/trn_rl_repo")

BS, E, TOPK, PL, D = 2048, 64, 8, 32, 512
NF = PL * D  # 16384 flattened prompt*dim
N_CORES = 8
RPC = BS // N_CORES  # 256 rows per core
RCHUNKS = RPC // 128  # 2 row chunks of 128
HALF = NF // 2  # 8192 output cols per half
WCHUNKS = 4  # W loaded in 4 chunks of [64, 4096]
WCW = HALF // WCHUNKS  # 2048
SLICES = WCW // D  # 4 matmuls (512 cols) per (chunk, rowchunk, half)
NPSUM = 7  # matmul PSUM ring (one PSUM bank each; ctp uses the 8th)

# aux tensor column layout (bf16): iota | ident | sc0 | sc1 | idx (fp32 bytes)
A_IOTA = 0
A_IDENT = A_IOTA + E  # 64
A_SC = A_IDENT + 128  # 192
A_IDX = A_SC + RCHUNKS * E  # 320 (idx stored as fp32 = 2 bf16 cols each)
A_COLS = A_IDX + RCHUNKS * TOPK * 2  # 352

_cache: dict = {}


def _build_program():
    import concourse.bass as bass
    import concourse.mybir as mybir

    f32 = mybir.dt.float32
    bf16 = mybir.dt.bfloat16
    nc = bass.Bass()

    aux_d = nc.declare_dram_parameter("aux", [128, A_COLS], bf16, isOutput=False)
    # W_flat [64, 16384] bf16 on partitions 0:64, columns c-major:
    # col c*4096 + h*2048 + s*512 holds output cols h*8192 + c*2048 + s*512.
    wk_d = nc.declare_dram_parameter("wk", [64, NF], bf16, isOutput=False)
    out_d = nc.declare_dram_parameter("out", [RPC, NF], bf16, isOutput=True)

    # matmul m (PE order) -> (wchunk c, rowchunk rc, half h, slice s)
    def mm_seq():
        m = 0
        for rc in range(RCHUNKS):
            for c in range(WCHUNKS):
                for h in range(2):
                    for s in range(SLICES):
                        yield m, c, rc, h, s
                        m += 1

    N_MM = WCHUNKS * RCHUNKS * 2 * SLICES  # 64
    mm_info = {m: (c, rc, h, s) for m, c, rc, h, s in mm_seq()}

    # s_pe increment index of each matmul / transpose (warmups don't inc;
    # T1 sits between m=7 and m=8).
    pe_idx: dict = {}
    n = 1
    pe_idx["T0"] = n
    for m in range(N_MM):
        if m == N_MM // 2:
            n += 1
            pe_idx["T1"] = n
        n += 1
        pe_idx[m] = n

    # Store list: (rc, colbase, width, [matmul indices]).  c0 groups are
    # 1024-col stores (2 slices); every other (c, rc, h) one 2048-col store.
    stores = []
    for rc in range(RCHUNKS):
        for c in range(WCHUNKS):
            base_m = (rc * WCHUNKS + c) * 2 * SLICES
            for h in range(2):
                cb = h * HALF + c * WCW
                m0 = base_m + h * SLICES
                stores.append((rc, cb, 1024, [m0, m0 + 1]))
                stores.append((rc, cb + 1024, 1024, [m0 + 2, m0 + 3]))

    # Copy assignment: slices alternate engines by matmul parity (even m ->
    # ACT, odd m -> DVE) so both engines advance every store.  copy_pos[m] =
    # (eng, 1-based position in that engine's copy stream); store_thr[g] =
    # per-engine wait thresholds for store g.
    copy_pos: dict = {}
    eng_seq: dict = {"v": [], "a": []}
    for g, (rc, cb, width, ms) in enumerate(stores):
        for m in ms:
            eng = "a" if m % 2 == 0 else "v"
            eng_seq[eng].append(m)
            copy_pos[m] = (eng, len(eng_seq[eng]))
    store_thr = []
    for rc, cb, width, ms in stores:
        thr: dict = {}
        for m in ms:
            eng, pos = copy_pos[m]
            thr[eng] = max(thr.get(eng, 0), pos)
        store_thr.append(thr)

    ctx = ExitStack()
    with ctx:
        sb = lambda shape, tag, dt=bf16: ctx.enter_context(  # noqa: E731
            nc.sbuf_tensor(tag, shape, dt)
        )
        aux_t = sb([128, A_COLS], "aux_t")
        w_t = sb([64, NF], "w_t")
        # rc0 (DVE) and rc1 (GPSIMD) count/score chains, all bf16
        eqs = [sb([128, E], f"eq{i}") for i in range(TOPK)]
        prs = [sb([128, E], f"pr{i}") for i in range(TOPK // 2)]
        qds = [sb([128, E], f"qd{i}") for i in range(TOPK // 4)]
        eqs2 = [sb([128, E], f"eqg{i}") for i in range(TOPK)]
        prs2 = [sb([128, E], f"prg{i}") for i in range(TOPK // 2)]
        qds2 = [sb([128, E], f"qdg{i}") for i in range(TOPK // 4)]
        cnt = [sb([128, E], f"cnt{r}") for r in range(RCHUNKS)]
        ct = [sb([64, 128], f"ct{r}") for r in range(RCHUNKS)]
        # staging: one [128, 4096] bf16 tensor per (c, rc) block
        stg = [sb([128, 2 * WCW], f"stg{g}") for g in range(WCHUNKS * RCHUNKS)]
        scr_a = sb([128, 128], "scr_a")
        scr_s = sb([128, D], "scr_s")

        ctp_all = ctx.enter_context(nc.psum_tensor("ctp", [E, 256], bf16))
        ctp = [ctp_all[:, r * 128 : (r + 1) * 128] for r in range(RCHUNKS)]
        pmm = [
            ctx.enter_context(nc.psum_tensor(f"pmm{i}", [128, D], f32))
            for i in range(NPSUM)
        ]

        s_in = ctx.enter_context(nc.semaphore("s_in"))
        s_w = [
            ctx.enter_context(nc.semaphore(f"s_w{c}")) for c in range(WCHUNKS)
        ]
        s_dve = ctx.enter_context(nc.semaphore("s_dve"))
        s_gp = ctx.enter_context(nc.semaphore("s_gp"))
        s_ct0 = ctx.enter_context(nc.semaphore("s_ct0"))
        s_ct1 = ctx.enter_context(nc.semaphore("s_ct1"))
        s_pe = ctx.enter_context(nc.semaphore("s_pe"))
        s_act = ctx.enter_context(nc.semaphore("s_act"))
        s_cpv = ctx.enter_context(nc.semaphore("s_cpv"))
        s_out = ctx.enter_context(nc.semaphore("s_out"))
        sem_of = {"v": s_cpv, "a": s_act}

        ident = lambda: aux_t[:, A_IDENT : A_IDENT + 128]  # noqa: E731
        iota_f = lambda: aux_t[:, A_IOTA : A_IOTA + E]  # noqa: E731
        sc = lambda r: aux_t[:, A_SC + r * E : A_SC + (r + 1) * E]  # noqa: E731
        idxcol = lambda r, k: aux_t[  # noqa: E731
            :, A_IDX + 2 * (r * TOPK + k) : A_IDX + 2 * (r * TOPK + k) + 2
        ].bitcast(f32)
        sgi = lambda c, rc: c * RCHUNKS + rc  # staging index  # noqa: E731

        def stg_sl(m):
            c, rc, h, s = mm_info[m]
            col = (h * SLICES + s) * D
            return stg[sgi(c, rc)][:, col : col + D]

        block = ctx.enter_context(nc.Block())

        @block.sync
        def _(sp):
            sp.dma_start(out=aux_t[:], in_=aux_d[:]).then_inc(s_in, 16)
            for c in range(WCHUNKS):
                cols = slice(c * 2 * WCW, (c + 1) * 2 * WCW)
                sp.dma_start(out=w_t[:, cols], in_=wk_d[:, cols]).then_inc(
                    s_w[c], 16
                )
            for g, (rc, cb, width, ms) in enumerate(stores):
                rows = slice(rc * 128, (rc + 1) * 128)
                c = (cb % HALF) // WCW
                off = cb % WCW + (cb // HALF) * WCW
                waits = list(store_thr[g].items())
                for eng, pos in waits[:-1]:
                    sp.wait_ge(sem_of[eng], pos)
                # last wait rides on the DMA descriptor itself (one sync
                # wait per instruction), saving a standalone wait per store
                sp.dma_start(
                    out=out_d[rows, cb : cb + width],
                    in_=stg[sgi(c, rc)][:, off : off + width],
                )._wait_ge(sem_of[waits[-1][0]], waits[-1][1]).then_inc(s_out, 16)

        @block.vector
        def _(v):
            v.wait_ge(s_in, 16)
            for k in range(TOPK):
                v.tensor_scalar(
                    eqs[k][:], iota_f(), idxcol(0, k), None,
                    mybir.AluOpType.is_equal,
                )
            v.drain()
            for i in range(TOPK // 2):
                v.tensor_add(prs[i][:], eqs[2 * i][:], eqs[2 * i + 1][:])
            v.drain()
            for i in range(TOPK // 4):
                v.tensor_add(qds[i][:], prs[2 * i][:], prs[2 * i + 1][:])
            v.drain()
            v.tensor_add(cnt[0][:], qds[0][:], qds[1][:])
            v.drain()
            v.tensor_mul(cnt[0][:], cnt[0][:], sc(0)).then_inc(s_dve, 1)
            v.wait_ge(s_pe, pe_idx["T0"])
            v.tensor_copy(ct[0][:], ctp[0][:]).then_inc(s_ct0, 1)
            ct1_done = False
            for m in eng_seq["v"]:
                if m >= N_MM // 2 and not ct1_done:
                    v.wait_ge(s_pe, pe_idx["T1"])
                    v.tensor_copy(ct[1][:], ctp[1][:]).then_inc(s_ct1, 1)
                    ct1_done = True
                v.wait_ge(s_pe, pe_idx[m])
                v.tensor_copy(stg_sl(m), pmm[m % NPSUM][:]).then_inc(s_cpv, 1)

        @block.scalar
        def _(a):
            for m in eng_seq["a"]:
                a.wait_ge(s_pe, pe_idx[m])
                a.copy(stg_sl(m), pmm[m % NPSUM][:]).then_inc(s_act, 1)

        @block.gpsimd
        def _(gp):
            gp.memset(scr_a[:], 0)
            gp.memset(scr_s[:], 0).then_inc(s_gp, 1)
            gp.wait_ge(s_in, 16)
            for k in range(TOPK):
                gp.tensor_scalar(
                    eqs2[k][:], iota_f(), idxcol(1, k), None,
                    mybir.AluOpType.is_equal,
                )
            gp.drain()
            for i in range(TOPK // 2):
                gp.tensor_add(prs2[i][:], eqs2[2 * i][:], eqs2[2 * i + 1][:])
            gp.drain()
            for i in range(TOPK // 4):
                gp.tensor_add(qds2[i][:], prs2[2 * i][:], prs2[2 * i + 1][:])
            gp.drain()
            gp.tensor_add(cnt[1][:], qds2[0][:], qds2[1][:])
            gp.drain()
            gp.tensor_mul(cnt[1][:], cnt[1][:], sc(1)).then_inc(s_gp, 1)  # -> 2

        @block.tensor
        def _(t):
            t.wait_ge(s_gp, 1)  # scratch memset done
            t.matmul(
                pmm[NPSUM - 1][:], scr_a[:E, :], scr_s[:E, :],
                start=True, stop=True,
            )
            t.wait_ge(s_in, 16)  # ident (aux); also paces the p-state ramp:
            # idle gaps > 3us reset pe_busy_start, so a second warmup here
            # (~3.4us) keeps every later gap under 3us and the real matmuls
            # at full clock.
            t.matmul(
                pmm[NPSUM - 1][:], scr_a[:E, :], scr_s[:E, :],
                start=True, stop=True,
            )
            t.wait_ge(s_dve, 1)
            t.transpose(ctp[0][:], cnt[0][:], ident()).then_inc(s_pe, 1)
            t.wait_ge(s_ct0, 1)  # ct0 copy done
            cur_c = -1
            for m, c, rc, h, s in mm_seq():
                if m == N_MM // 2:
                    t.wait_ge(s_gp, 2)
                    t.transpose(ctp[1][:], cnt[1][:], ident()).then_inc(s_pe, 1)
                    t.wait_ge(s_ct1, 1)  # ct1 copy done
                if c != cur_c:
                    t.wait_ge(s_w[c], 16)
                    cur_c = c
                if m >= NPSUM:
                    eng, pos = copy_pos[m - NPSUM]
                    t.wait_ge(sem_of[eng], pos)
                wc = c * 2 * WCW + h * WCW + s * D
                t.matmul(
                    pmm[m % NPSUM][:],
                    ct[rc][:],
                    w_t[:, wc : wc + D],
                    start=True,
                    stop=True,
                ).then_inc(s_pe, 1)

    return nc


def _run(selection_score, expert_indices, all_weight, trace=False):
    import ml_dtypes
    from concourse.bass_utils import run_bass_kernel_spmd

    bf16 = ml_dtypes.bfloat16
    scores = np.asarray(selection_score, dtype=np.float32)
    idxf = np.asarray(expert_indices).astype(np.float32)
    w = np.asarray(all_weight, dtype=np.float32).reshape(E, NF)
    # [e, h, c, 2048] -> [e, c, h, 2048] so each W-chunk DMA is contiguous
    wk = np.ascontiguousarray(
        w.reshape(E, 2, WCHUNKS, WCW).transpose(0, 2, 1, 3).reshape(E, NF).astype(bf16)
    )
    iota = np.tile(np.arange(E, dtype=np.float32), (128, 1))
    ident = np.eye(128, dtype=np.float32)

    if "nc" not in _cache:
        _cache["nc"] = _build_program()
    nc = _cache["nc"]

    in_maps = []
    for c in range(N_CORES):
        rows = slice(c * RPC, (c + 1) * RPC)
        scb = scores[rows].reshape(RCHUNKS, 128, E)
        ix = idxf[rows].reshape(RCHUNKS, 128, TOPK)
        aux = np.concatenate(
            [iota, ident, scb[0], scb[1]], axis=1, dtype=np.float32
        ).astype(bf16)
        # idx values stay fp32, byte-spliced into the bf16 tensor (device
        # bitcasts the 2-col pairs back to fp32 scalars)
        idx_bytes = np.concatenate([ix[0], ix[1]], axis=1, dtype=np.float32)
        aux = np.concatenate([aux, idx_bytes.view(bf16)], axis=1)
        in_maps.append({"aux": np.ascontiguousarray(aux), "wk": wk})
    r = run_bass_kernel_spmd(nc, in_maps, list(range(N_CORES)), trace=trace)
    full = np.concatenate(
        [np.asarray(r.results[c]["out"]).astype(np.float32) for c in range(N_CORES)],
        axis=0,
    )
    return full.reshape(BS, PL, D), r


def kernel(selection_score, expert_indices, all_weight) -> np.ndarray:
    full, _ = _run(selection_score, expert_indices, all_weight, trace=False)
    return full
